# revision 16
# baseline (speedup 1.0000x reference)
"""Trainium2 Bass kernel for the Elman-RNN place-cell problem.

Strategy: tensor-parallel over the hidden dimension NG=4096 across 8 cores.
Each core keeps a [4096, 512] column-shard of W_rec resident in SBUF and
computes its 512-column shard of h_{t+1} = relu(x_t + h_t @ W_rec) for the
full batch B=256 each step; a per-step AllGather rebuilds the full hidden
state (transposed layout [NG, B]) on every core.  The decode matmul
(g @ W_dec) is split across cores by NP columns (64 each) and runs on the
TensorE during the AllGather wait.  The encoder (P0 @ W_enc) and the input
projection (v @ W_in) are tiny (<2% FLOPs) and are done on the host as part
of input sharding.
"""
import os
import sys
import functools

sys.path.insert(0, "/opt/trn_rl_repo")

import numpy as np

from concourse import bass, bacc, mybir, tile  # noqa: E402
from concourse import bass_utils  # noqa: E402

B = 256
T = 100
NG = 4096
NP = 512
NCORES = 8
S = NG // NCORES          # 512 hidden columns per core
KT = NG // 128            # 32 contraction tiles
MT = S // 128             # 4 output tiles per core shard
NPS = NP // NCORES        # 64 decode columns per core
FP = mybir.dt.float32

# compute dtype for matmul-facing tensors: "f32r" (fp32 w/ 11-bit mantissa,
# full-rate PE), "f32" (exact, 4x slower PE), or "bf16"
CDTYPE = os.environ.get("RNN_CDTYPE", "f32r")


def _cd():
    return {"f32": mybir.dt.float32,
            "f32r": mybir.dt.float32r,
            "bf16": mybir.dt.bfloat16}[CDTYPE]


# one AllGather per batch-half per step; k-tiles stay in natural order
K_ORDER = list(range(KT))


def _build(t_steps=T):
    CD = _cd()
    nc = bacc.Bacc("TRN2", target_bir_lowering=False, debug=False,
                   num_devices=NCORES)
    wrec = nc.dram_tensor("wrec", [128, KT * S], CD, kind="ExternalInput")
    # decode weights stay exact fp32 when compute dtype allows a fp32 view
    DDT = FP if CDTYPE in ("f32", "f32r") else CD
    wdec = nc.dram_tensor("wdec", [128, KT * NPS], DDT, kind="ExternalInput")
    xin = nc.dram_tensor("x", [t_steps, S, B], FP, kind="ExternalInput")
    h0 = nc.dram_tensor("h0", [NG, B], CD, kind="ExternalInput")
    out = nc.dram_tensor("out", [t_steps, 2, 128, NPS], FP,
                         kind="ExternalOutput")

    with tile.TileContext(nc) as tc:
        with tc.tile_pool(name="wpool", bufs=1) as wpool, \
             tc.tile_pool(name="hpool", bufs=2) as hpool, \
             tc.tile_pool(name="xpool", bufs=3) as xpool, \
             tc.tile_pool(name="hnpool", bufs=2) as hnpool, \
             tc.tile_pool(name="decpool", bufs=3) as decpool, \
             tc.tile_pool(name="psr", bufs=1, space="PSUM") as psr, \
             tc.tile_pool(name="psd", bufs=2, space="PSUM") as psd, \
             tc.tile_pool(name="dram_i", bufs=3, space="DRAM") as dram_i, \
             tc.tile_pool(name="dram_o", bufs=3, space="DRAM") as dram_o:

            wrec_sb = wpool.tile([128, KT * S], CD, name="wrec_sb")
            nc.scalar.dma_start(out=wrec_sb[:], in_=wrec[:])
            wdec_sb = wpool.tile([128, KT * NPS], DDT, name="wdec_sb")
            nc.scalar.dma_start(out=wdec_sb[:], in_=wdec[:])

            # The batch is split in two independent halves (BH=128 each);
            # their recurrences interleave so one half's AllGather hides
            # under the other half's matmuls.  h state lives in BCH=4
            # quarter tiles per half so the post-AllGather bounce DMAs land
            # piecewise and matmuls start on quarter 0.
            BCH = 4
            KPB = KT // BCH
            BH = B // 2

            def new_h_tiles(h):
                return [hpool.tile([128, KPB, BH], CD, name=f"hT{h}_{c}",
                                   tag=f"hT{h}_{c}") for c in range(BCH)]

            hts = [new_h_tiles(0), new_h_tiles(1)]
            for h in range(2):
                for c in range(BCH):
                    nc.sync.dma_start(
                        out=hts[h][c][:],
                        in_=h0[c * KPB * 128:(c + 1) * KPB * 128,
                               h * BH:(h + 1) * BH]
                        .rearrange("(k p) b -> p k b", p=128))

            def decode(srcs, h, t):
                # out[t, h*128+p, r*NPS+f] = (g_t @ W_dec)[h*128+p, shard]
                dec_sb = decpool.tile([128, NPS], FP, name="dec_sb")
                ps = psd.tile([128, NPS], FP, name="ps_dec")
                for k in range(KT):
                    lhsT = srcs[k // KPB][:, k % KPB, :]
                    if CDTYPE == "f32r":
                        lhsT = lhsT.bitcast(FP)
                    nc.tensor.matmul(
                        ps[:],
                        lhsT,
                        wdec_sb[:, k * NPS:(k + 1) * NPS],
                        start=(k == 0), stop=(k == KT - 1))
                nc.vector.tensor_copy(dec_sb[:], ps[:])
                nc.scalar.dma_start(out=out[t, h], in_=dec_sb[:])

            rg = [list(range(NCORES))]
            for i in range(t_steps):
                for h in range(2):
                    x_t = xpool.tile([128, MT, BH], FP, name=f"x_t{h}",
                                     tag=f"x_t{h}")
                    nc.scalar.dma_start(
                        out=x_t[:],
                        in_=xin[i][:, h * BH:(h + 1) * BH]
                        .rearrange("(m p) b -> p m b", p=128))

                    # two m-banks share one PSUM bank (padded per tile)
                    pss = [psr.tile([128, 2, BH], FP, name=f"ps{h}_{p}",
                                    tag=f"ps{h}_{p}") for p in range(2)]

                    hn = hnpool.tile([128, MT, BH], CD, name=f"hn{h}",
                                     tag=f"hn{h}")
                    cc_i = dram_i.tile([S, BH], CD, name=f"cc_i{h}",
                                       tag=f"cc_i{h}")
                    cc_o = dram_o.tile([NG, BH], CD, name=f"cc_o{h}",
                                       tag=f"cc_o{h}", addr_space="Shared")

                    # quarters 0..BCH-2 k-outer (start as each lands)
                    for c in range(BCH - 1):
                        for kl in range(KPB):
                            k = c * KPB + kl
                            for m in range(MT):
                                # start=True zeroes the whole 2KB bank, so
                                # only the pair's first m-group issues it
                                nc.tensor.matmul(
                                    pss[m // 2][:, m % 2, :],
                                    wrec_sb[:, k * S + m * 128:
                                            k * S + (m + 1) * 128],
                                    hts[h][c][:, kl, :],
                                    start=(k == 0 and m % 2 == 0), stop=False,
                                    skip_group_check=True)

                    # last quarter m-outer: close banks early, send asap
                    c = BCH - 1
                    for m in range(MT):
                        for kl in range(KPB):
                            k = c * KPB + kl
                            nc.tensor.matmul(
                                pss[m // 2][:, m % 2, :],
                                wrec_sb[:, k * S + m * 128:
                                        k * S + (m + 1) * 128],
                                hts[h][c][:, kl, :],
                                start=False, stop=(kl == KPB - 1),
                                skip_group_check=True)
                        nc.vector.tensor_tensor(hn[:, m, :],
                                                pss[m // 2][:, m % 2, :],
                                                x_t[:, m, :],
                                                mybir.AluOpType.add)
                        nc.vector.tensor_scalar_max(hn[:, m, :],
                                                    hn[:, m, :], 0.0)
                    nc.sync.dma_start(
                        out=cc_i[:].rearrange("(m p) b -> p m b", p=128),
                        in_=hn[:])
                    nc.gpsimd.collective_compute(
                        "AllGather", mybir.AluOpType.bypass,
                        replica_groups=rg,
                        ins=[cc_i[:].opt()], outs=[cc_o[:].opt()])

                    # decode of this half's previous state fills the AG wait
                    if i >= 1:
                        decode(hts[h], h, i - 1)

                    hts[h] = new_h_tiles(h)
                    for c in range(BCH):
                        nc.sync.dma_start(
                            out=hts[h][c][:],
                            in_=cc_o[c * KPB * 128:(c + 1) * KPB * 128]
                            .rearrange("(k p) b -> p k b", p=128))

            for h in range(2):
                decode(hts[h], h, t_steps - 1)

    nc.compile()
    return nc


@functools.lru_cache(maxsize=1)
def _built():
    return _build()


def _round_f32r(a):
    """Round fp32 to the PE's FP32r format (11-bit mantissa, RNE)."""
    u = np.ascontiguousarray(a, np.float32).view(np.uint32)
    r = (u.astype(np.uint64) + 0x7FF + ((u >> 12) & 1)).astype(np.uint32)
    return (r & np.uint32(0xFFFFF000)).view(np.float32)


def _to_cd(a):
    if CDTYPE == "f32r":
        return _round_f32r(a)
    if CDTYPE == "bf16":
        import ml_dtypes
        return np.ascontiguousarray(a).astype(ml_dtypes.bfloat16)
    return np.ascontiguousarray(a, np.float32)


def _prep_inputs(v, P0, W_enc, W_in, W_rec, W_dec, t_steps=T):
    v = np.asarray(v, np.float32)
    P0 = np.asarray(P0, np.float32)
    W_enc = np.asarray(W_enc, np.float32)
    W_in = np.asarray(W_in, np.float32)
    W_rec = np.asarray(W_rec, np.float32)
    W_dec = np.asarray(W_dec, np.float32)

    # x[t, b, g] = sum_d v[b, t, d] W_in[d, g]; stored transposed [T, NG, B]
    x = (v.reshape(-1, v.shape[-1]) @ W_in).reshape(B, T, NG)
    xT = np.ascontiguousarray(x.transpose(1, 2, 0))  # [T, NG, B]
    h0T = np.ascontiguousarray((P0 @ W_enc).T)       # [NG, B]

    # pack contraction k-tiles in the chunked AllGather order
    wrec_r = W_rec.reshape(KT, 128, NG)[K_ORDER]
    wdec_r = W_dec.reshape(KT, 128, NP)[K_ORDER]
    h0T = np.ascontiguousarray(
        h0T.reshape(KT, 128, B)[K_ORDER].reshape(NG, B))

    in_maps = []
    for r in range(NCORES):
        wrec_core = np.ascontiguousarray(
            wrec_r[:, :, r * S:(r + 1) * S].transpose(1, 0, 2)
        ).reshape(128, KT * S)
        wdec_core = np.ascontiguousarray(
            wdec_r[:, :, r * NPS:(r + 1) * NPS].transpose(1, 0, 2)
        ).reshape(128, KT * NPS)
        x_core = np.ascontiguousarray(xT[:t_steps, r * S:(r + 1) * S, :])
        in_maps.append({
            "wrec": _to_cd(wrec_core),
            # decode weights stay exact fp32 for f32/f32r compute modes
            "wdec": (wdec_core if CDTYPE in ("f32", "f32r")
                     else _to_cd(wdec_core)),
            "x": x_core,
            "h0": _to_cd(h0T),
        })
    return in_maps


def _assemble(results, t_steps=T):
    full = np.empty((B, t_steps, NP), np.float32)
    for r in range(NCORES):
        a = results[r]["out"]  # [t_steps, 2, 128, NPS]
        full[:, :, r * NPS:(r + 1) * NPS] = \
            a.reshape(t_steps, B, NPS).transpose(1, 0, 2)
    return full


last_exec_time_ns = None


def kernel(v, P0, W_enc, W_in, W_rec, W_dec):
    global last_exec_time_ns
    nc = _built()
    in_maps = _prep_inputs(v, P0, W_enc, W_in, W_rec, W_dec)

    trace = bool(int(os.environ.get("RNN_TRACE", "0")))
    if trace:
        # NTFF profiling hook (the image's antenv lacks axon_hooks; shim it).
        try:
            import types
            sys.path.insert(0, "/root/.axon_site")
            from trn_agent_boot.trn_boot import _ntff_profile_via_ctypes
            import antenv  # noqa: F401
            if "antenv.axon_hooks" not in sys.modules:
                mod = types.ModuleType("antenv.axon_hooks")
                hook = _ntff_profile_via_ctypes("/opt/axon/libaxon_pjrt.so")
                mod.get_axon_ntff_profile_hook = lambda: hook
                sys.modules["antenv.axon_hooks"] = mod
        except Exception as e:  # pragma: no cover
            print("trace shim failed:", e)

    res = bass_utils.run_bass_kernel_spmd(
        nc, in_maps, core_ids=list(range(NCORES)), trace=trace)
    last_exec_time_ns = res.exec_time_ns
    return _assemble(res.results)


# revision 18
# speedup vs baseline: 2.1923x; 2.1923x over previous
"""Trainium2 Bass kernel for the Elman-RNN place-cell problem.

Strategy: tensor-parallel over the hidden dimension NG=4096 across 8 cores.
Each core keeps a [4096, 512] column-shard of W_rec resident in SBUF and
computes its 512-column shard of h_{t+1} = relu(x_t + h_t @ W_rec) for the
full batch B=256 each step; a per-step AllGather rebuilds the full hidden
state (transposed layout [NG, B]) on every core.  The decode matmul
(g @ W_dec) is split across cores by NP columns (64 each) and runs on the
TensorE during the AllGather wait.  The encoder (P0 @ W_enc) and the input
projection (v @ W_in) are tiny (<2% FLOPs) and are done on the host as part
of input sharding.
"""
import os
import sys
import functools

sys.path.insert(0, "/opt/trn_rl_repo")

import numpy as np

from concourse import bass, bacc, mybir, tile  # noqa: E402
from concourse import bass_utils  # noqa: E402

B = 256
T = 100
NG = 4096
NP = 512
NCORES = 8
S = NG // NCORES          # 512 hidden columns per core
KT = NG // 128            # 32 contraction tiles
MT = S // 128             # 4 output tiles per core shard
NPS = NP // NCORES        # 64 decode columns per core
FP = mybir.dt.float32

# compute dtype for matmul-facing tensors: "bf16" (fastest), "f32r" (fp32
# w/ 11-bit mantissa; needs N>=256 so only sensible without batch-split),
# or "f32" (exact, 4x slower PE)
CDTYPE = os.environ.get("RNN_CDTYPE", "bf16")


def _cd():
    return {"f32": mybir.dt.float32,
            "f32r": mybir.dt.float32r,
            "bf16": mybir.dt.bfloat16}[CDTYPE]


# one AllGather per batch-half per step; k-tiles stay in natural order
K_ORDER = list(range(KT))


def _build(t_steps=T):
    CD = _cd()
    nc = bacc.Bacc("TRN2", target_bir_lowering=False, debug=False,
                   num_devices=NCORES)
    wrec = nc.dram_tensor("wrec", [128, KT * S], CD, kind="ExternalInput")
    # decode weights stay exact fp32 when compute dtype allows a fp32 view
    DDT = FP if CDTYPE in ("f32", "f32r") else CD
    wdec = nc.dram_tensor("wdec", [128, KT * NPS], DDT, kind="ExternalInput")
    xin = nc.dram_tensor("x", [t_steps, S, B], FP, kind="ExternalInput")
    h0 = nc.dram_tensor("h0", [NG, B], CD, kind="ExternalInput")
    out = nc.dram_tensor("out", [t_steps, 2, 128, NPS], FP,
                         kind="ExternalOutput")

    with tile.TileContext(nc) as tc:
        with tc.tile_pool(name="wpool", bufs=1) as wpool, \
             tc.tile_pool(name="hpool", bufs=2) as hpool, \
             tc.tile_pool(name="xpool", bufs=3) as xpool, \
             tc.tile_pool(name="hnpool", bufs=2) as hnpool, \
             tc.tile_pool(name="decpool", bufs=3) as decpool, \
             tc.tile_pool(name="psr", bufs=1, space="PSUM") as psr, \
             tc.tile_pool(name="psd", bufs=2, space="PSUM") as psd, \
             tc.tile_pool(name="dram_i", bufs=3, space="DRAM") as dram_i, \
             tc.tile_pool(name="dram_o", bufs=3, space="DRAM") as dram_o:

            wrec_sb = wpool.tile([128, KT * S], CD, name="wrec_sb")
            nc.scalar.dma_start(out=wrec_sb[:], in_=wrec[:])
            wdec_sb = wpool.tile([128, KT * NPS], DDT, name="wdec_sb")
            nc.scalar.dma_start(out=wdec_sb[:], in_=wdec[:])

            # The batch is split in two independent halves (BH=128 each);
            # their recurrences interleave so one half's AllGather hides
            # under the other half's matmuls.  h state lives in BCH=4
            # quarter tiles per half so the post-AllGather bounce DMAs land
            # piecewise and matmuls start on quarter 0.
            BCH = 4
            KPB = KT // BCH
            BH = B // 2

            def new_h_tiles(h):
                return [hpool.tile([128, KPB, BH], CD, name=f"hT{h}_{c}",
                                   tag=f"hT{h}_{c}") for c in range(BCH)]

            hts = [new_h_tiles(0), new_h_tiles(1)]
            for h in range(2):
                for c in range(BCH):
                    nc.sync.dma_start(
                        out=hts[h][c][:],
                        in_=h0[c * KPB * 128:(c + 1) * KPB * 128,
                               h * BH:(h + 1) * BH]
                        .rearrange("(k p) b -> p k b", p=128))

            def decode(srcs, h, t):
                # out[t, h*128+p, r*NPS+f] = (g_t @ W_dec)[h*128+p, shard]
                dec_sb = decpool.tile([128, NPS], FP, name="dec_sb")
                ps = psd.tile([128, NPS], FP, name="ps_dec")
                for k in range(KT):
                    lhsT = srcs[k // KPB][:, k % KPB, :]
                    if CDTYPE == "f32r":
                        lhsT = lhsT.bitcast(FP)
                    nc.tensor.matmul(
                        ps[:],
                        lhsT,
                        wdec_sb[:, k * NPS:(k + 1) * NPS],
                        start=(k == 0), stop=(k == KT - 1))
                nc.vector.tensor_copy(dec_sb[:], ps[:])
                nc.scalar.dma_start(out=out[t, h], in_=dec_sb[:])

            rg = [list(range(NCORES))]
            for i in range(t_steps):
                for h in range(2):
                    x_t = xpool.tile([128, MT, BH], FP, name=f"x_t{h}",
                                     tag=f"x_t{h}")
                    nc.scalar.dma_start(
                        out=x_t[:],
                        in_=xin[i][:, h * BH:(h + 1) * BH]
                        .rearrange("(m p) b -> p m b", p=128))

                    # two m-banks share one PSUM bank (padded per tile)
                    pss = [psr.tile([128, 2, BH], FP, name=f"ps{h}_{p}",
                                    tag=f"ps{h}_{p}") for p in range(2)]

                    hn = hnpool.tile([128, MT, BH], CD, name=f"hn{h}",
                                     tag=f"hn{h}")
                    cc_i = dram_i.tile([S, BH], CD, name=f"cc_i{h}",
                                       tag=f"cc_i{h}")
                    cc_o = dram_o.tile([NG, BH], CD, name=f"cc_o{h}",
                                       tag=f"cc_o{h}", addr_space="Shared")

                    # quarters 0..BCH-2 k-outer (start as each lands)
                    for c in range(BCH - 1):
                        for kl in range(KPB):
                            k = c * KPB + kl
                            for m in range(MT):
                                # start=True zeroes the whole 2KB bank, so
                                # only the pair's first m-group issues it
                                nc.tensor.matmul(
                                    pss[m // 2][:, m % 2, :],
                                    wrec_sb[:, k * S + m * 128:
                                            k * S + (m + 1) * 128],
                                    hts[h][c][:, kl, :],
                                    start=(k == 0 and m % 2 == 0), stop=False,
                                    skip_group_check=True)

                    # last quarter m-outer: close banks early, send asap
                    c = BCH - 1
                    for m in range(MT):
                        for kl in range(KPB):
                            k = c * KPB + kl
                            nc.tensor.matmul(
                                pss[m // 2][:, m % 2, :],
                                wrec_sb[:, k * S + m * 128:
                                        k * S + (m + 1) * 128],
                                hts[h][c][:, kl, :],
                                start=False, stop=(kl == KPB - 1),
                                skip_group_check=True)
                        nc.vector.tensor_tensor(hn[:, m, :],
                                                pss[m // 2][:, m % 2, :],
                                                x_t[:, m, :],
                                                mybir.AluOpType.add)
                        nc.vector.tensor_scalar_max(hn[:, m, :],
                                                    hn[:, m, :], 0.0)
                    nc.sync.dma_start(
                        out=cc_i[:].rearrange("(m p) b -> p m b", p=128),
                        in_=hn[:])
                    nc.gpsimd.collective_compute(
                        "AllGather", mybir.AluOpType.bypass,
                        replica_groups=rg,
                        ins=[cc_i[:].opt()], outs=[cc_o[:].opt()])

                    # decode of this half's previous state fills the AG wait
                    if i >= 1:
                        decode(hts[h], h, i - 1)

                    hts[h] = new_h_tiles(h)
                    for c in range(BCH):
                        nc.sync.dma_start(
                            out=hts[h][c][:],
                            in_=cc_o[c * KPB * 128:(c + 1) * KPB * 128]
                            .rearrange("(k p) b -> p k b", p=128))

            for h in range(2):
                decode(hts[h], h, t_steps - 1)

    nc.compile()
    return nc


@functools.lru_cache(maxsize=1)
def _built():
    return _build()


def _round_f32r(a):
    """Round fp32 to the PE's FP32r format (11-bit mantissa, RNE)."""
    u = np.ascontiguousarray(a, np.float32).view(np.uint32)
    r = (u.astype(np.uint64) + 0x7FF + ((u >> 12) & 1)).astype(np.uint32)
    return (r & np.uint32(0xFFFFF000)).view(np.float32)


def _to_cd(a):
    if CDTYPE == "f32r":
        return _round_f32r(a)
    if CDTYPE == "bf16":
        import ml_dtypes
        return np.ascontiguousarray(a).astype(ml_dtypes.bfloat16)
    return np.ascontiguousarray(a, np.float32)


def _prep_inputs(v, P0, W_enc, W_in, W_rec, W_dec, t_steps=T):
    v = np.asarray(v, np.float32)
    P0 = np.asarray(P0, np.float32)
    W_enc = np.asarray(W_enc, np.float32)
    W_in = np.asarray(W_in, np.float32)
    W_rec = np.asarray(W_rec, np.float32)
    W_dec = np.asarray(W_dec, np.float32)

    # x[t, b, g] = sum_d v[b, t, d] W_in[d, g]; stored transposed [T, NG, B]
    x = (v.reshape(-1, v.shape[-1]) @ W_in).reshape(B, T, NG)
    xT = np.ascontiguousarray(x.transpose(1, 2, 0))  # [T, NG, B]
    h0T = np.ascontiguousarray((P0 @ W_enc).T)       # [NG, B]

    # pack contraction k-tiles in the chunked AllGather order
    wrec_r = W_rec.reshape(KT, 128, NG)[K_ORDER]
    wdec_r = W_dec.reshape(KT, 128, NP)[K_ORDER]
    h0T = np.ascontiguousarray(
        h0T.reshape(KT, 128, B)[K_ORDER].reshape(NG, B))

    in_maps = []
    for r in range(NCORES):
        wrec_core = np.ascontiguousarray(
            wrec_r[:, :, r * S:(r + 1) * S].transpose(1, 0, 2)
        ).reshape(128, KT * S)
        wdec_core = np.ascontiguousarray(
            wdec_r[:, :, r * NPS:(r + 1) * NPS].transpose(1, 0, 2)
        ).reshape(128, KT * NPS)
        x_core = np.ascontiguousarray(xT[:t_steps, r * S:(r + 1) * S, :])
        in_maps.append({
            "wrec": _to_cd(wrec_core),
            # decode weights stay exact fp32 for f32/f32r compute modes
            "wdec": (wdec_core if CDTYPE in ("f32", "f32r")
                     else _to_cd(wdec_core)),
            "x": x_core,
            "h0": _to_cd(h0T),
        })
    return in_maps


def _assemble(results, t_steps=T):
    full = np.empty((B, t_steps, NP), np.float32)
    for r in range(NCORES):
        a = results[r]["out"]  # [t_steps, 2, 128, NPS]
        full[:, :, r * NPS:(r + 1) * NPS] = \
            a.reshape(t_steps, B, NPS).transpose(1, 0, 2)
    return full


last_exec_time_ns = None


def kernel(v, P0, W_enc, W_in, W_rec, W_dec):
    global last_exec_time_ns
    nc = _built()
    in_maps = _prep_inputs(v, P0, W_enc, W_in, W_rec, W_dec)

    trace = bool(int(os.environ.get("RNN_TRACE", "0")))
    if trace:
        # NTFF profiling hook (the image's antenv lacks axon_hooks; shim it).
        try:
            import types
            sys.path.insert(0, "/root/.axon_site")
            from trn_agent_boot.trn_boot import _ntff_profile_via_ctypes
            import antenv  # noqa: F401
            if "antenv.axon_hooks" not in sys.modules:
                mod = types.ModuleType("antenv.axon_hooks")
                hook = _ntff_profile_via_ctypes("/opt/axon/libaxon_pjrt.so")
                mod.get_axon_ntff_profile_hook = lambda: hook
                sys.modules["antenv.axon_hooks"] = mod
        except Exception as e:  # pragma: no cover
            print("trace shim failed:", e)

    # the axon device occasionally reports a transient unrecoverable error
    # on the first execution of a fresh NEFF; retry a couple of times
    last_err = None
    for _ in range(3):
        try:
            res = bass_utils.run_bass_kernel_spmd(
                nc, in_maps, core_ids=list(range(NCORES)), trace=trace)
            last_exec_time_ns = res.exec_time_ns
            return _assemble(res.results)
        except Exception as e:  # pragma: no cover
            last_err = e
            import time
            time.sleep(5)
    raise last_err
